# revision 15
# baseline (speedup 1.0000x reference)
"""Multi-head causal attention (RoPE) Trainium2 kernel, SPMD over 8 NeuronCores.

Sharding: core c handles batch b = c // 4 and head-group g = c % 4
(4 heads of 128 dims each => 512 output features per core). Fully
embarrassingly parallel - no collectives.

Per-core device kernel (all matmuls bf16, f32 PSUM accumulation):
  - V projection first ([s, d] layout, bias via K=1 matmul, PSUM->SBUF
    evacuation on VectorE into a packed [128, 4*129] per-row-tile layout
    with a ones column per head so the softmax denominator falls out of
    the PV matmul for free).
  - Then per head: Q^T and K^T projections ([d, s] layout; bias added by
    ScalarE during evacuation; RoPE in half-rotated layout via a
    partition-half swap DMA + VectorE), followed by causal attention:
    scores^T = K^T-tile.T @ Q^T-chunk into paired [128, 1024] PSUM tiles,
    one big exp per pair on ScalarE (softmax scale folded in), narrowed
    diagonal tiles with one triangular 0/1 mask multiply on VectorE,
    PV accumulation, normalize on VectorE, DMA out.
  Head h's exp/mask work overlaps head h+1's projection matmuls, keeping
  TensorE the busy engine throughout.
"""

import sys

import numpy as np
import ml_dtypes

for _p in ("/opt/trn_rl_repo",):
    if _p not in sys.path:
        sys.path.insert(0, _p)

B, S, E = 2, 2048, 2048
H, D = 16, 128
P = 128
HPC = 4            # heads per core
F = HPC * D        # 512 projection features per core
NCORES = 8
NE = E // P        # 16 contraction tiles
NSQ = S // P       # 16 query row-tiles
NCH = S // 512     # 4 query chunks of 512
ROPE_BASE = 10000.0
SM_SCALE = 1.0 / float(np.sqrt(D))
BF16 = ml_dtypes.bfloat16

_compiled = None
LAST_RESULT = None

# interleaved (0,1),(2,3).. pairs -> half layout (i, i+64): new_i = old 2i,
# new_{i+64} = old 2i+1. Applied to q/k weight rows per head; scores are
# invariant since the same permutation hits q and k.
_PERM = np.concatenate([np.arange(0, D, 2), np.arange(1, D, 2)])


def _rope_tables():
    inv = ROPE_BASE ** (-np.arange(0, D, 2, dtype=np.float64) / D)      # [64]
    ang = np.arange(S, dtype=np.float64)[None, :] * inv[:, None]        # [64, S]
    cos, sin = np.cos(ang), np.sin(ang)
    cosf = np.concatenate([cos, cos], axis=0).astype(BF16)              # [128, S]
    ssin = np.concatenate([-sin, sin], axis=0).astype(BF16)
    return cosf, ssin


def _mask_tile():
    # mask[p, f] = 1 iff f >= p (lower-triangle-inclusive block)
    f = np.arange(512)[None, :]
    p = np.arange(P)[:, None]
    return (f >= p).astype(np.float32).astype(BF16)


def _build():
    import concourse.mybir as mybir
    import concourse.tile as tile
    from concourse import bacc

    fdt = mybir.dt.float32
    bdt = mybir.dt.bfloat16
    Exp = mybir.ActivationFunctionType.Exp
    Ident = mybir.ActivationFunctionType.Identity

    nc = bacc.Bacc("TRN2", target_bir_lowering=False, debug=False,
                   num_devices=NCORES)

    xt = nc.dram_tensor("xt", [E, S], bdt, kind="ExternalInput").ap()
    wqt = nc.dram_tensor("wqt", [HPC, E, D], bdt, kind="ExternalInput").ap()
    wkt = nc.dram_tensor("wkt", [HPC, E, D], bdt, kind="ExternalInput").ap()
    wvt = nc.dram_tensor("wvt", [E, F], bdt, kind="ExternalInput").ap()
    bqd = nc.dram_tensor("bqd", [P, HPC], fdt, kind="ExternalInput").ap()
    bkd = nc.dram_tensor("bkd", [P, HPC], fdt, kind="ExternalInput").ap()
    bvd = nc.dram_tensor("bvd", [1, F], bdt, kind="ExternalInput").ap()
    cosd = nc.dram_tensor("cosd", [P, S], bdt, kind="ExternalInput").ap()
    ssind = nc.dram_tensor("ssind", [P, S], bdt, kind="ExternalInput").ap()
    maskd = nc.dram_tensor("maskd", [P, 512], bdt, kind="ExternalInput").ap()
    outd = nc.dram_tensor("out", [S, F], fdt, kind="ExternalOutput").ap()

    with tile.TileContext(nc) as tc:
        with (
            tc.tile_pool(name="const", bufs=1) as constp,
            tc.tile_pool(name="persist", bufs=1) as persist,
            tc.tile_pool(name="psum", bufs=2, space="PSUM") as psp,
            tc.tile_pool(name="xtp", bufs=1) as xp,
            tc.tile_pool(name="wp", bufs=2) as wp,
            tc.tile_pool(name="evac", bufs=2) as ep,
            tc.tile_pool(name="etp", bufs=1) as etp,
            tc.tile_pool(name="etd", bufs=2) as etd,
            tc.tile_pool(name="ost", bufs=4) as osp,
        ):
            # small/late-needed constants on the scalar (ACT) HWDGE queue so
            # they don't delay the x^T / weight stream on the sync queue.
            bqpt = constp.tile([P, HPC], fdt, tag="bqpt", name="bqpt")
            nc.scalar.dma_start(bqpt[:], bqd[:])
            bkpt = constp.tile([P, HPC], fdt, tag="bkpt", name="bkpt")
            nc.scalar.dma_start(bkpt[:], bkd[:])
            cos_sb = constp.tile([P, S], bdt, tag="cos", name="cos_sb")
            nc.scalar.dma_start(cos_sb[:], cosd[:])
            ssin_sb = constp.tile([P, S], bdt, tag="ssin", name="ssin_sb")
            nc.scalar.dma_start(ssin_sb[:], ssind[:])
            mask_sb = constp.tile([P, 512], bdt, tag="mask", name="mask_sb")
            nc.scalar.dma_start(mask_sb[:], maskd[:])
            ones_row = constp.tile([1, P], bdt, tag="ones", name="ones_row")
            nc.vector.memset(ones_row[:], 1.0)

            qTt = persist.tile([P, S], bdt, tag="qTt", name="qTt")
            kTt = persist.tile([P, S], bdt, tag="kTt", name="kTt")
            vA = [persist.tile([P, HPC * (D + 1)], bdt, tag=f"vA{j}",
                               name=f"vA{j}") for j in range(NSQ)]

            # ---- input streams on the sync queue, in consumption order
            bv_sb = constp.tile([1, F], bdt, tag="bv", name="bv_sb")
            nc.sync.dma_start(bv_sb[:], bvd[:])
            xts = [None] * NE
            wvs = [None] * NE
            for e in range(NE):
                wtile = wp.tile([P, F], bdt, tag=f"wv{e}", name=f"v{e}",
                                bufs=1)
                nc.sync.dma_start(wtile[:], wvt[P * e:P * (e + 1), :])
                wvs[e] = wtile
                t = xp.tile([P, S], bdt, tag=f"x{e}", name=f"x{e}")
                nc.sync.dma_start(t[:], xt[P * e:P * (e + 1), :])
                xts[e] = t

            def load_wh(wd, h, pfx):
                # per-head just-in-time weight tiles ([HPC, E, D] in DRAM)
                wts = []
                for e in range(NE):
                    wtile = wp.tile([P, D], bdt, tag=f"wh{e}",
                                    name=f"{pfx}{h}_{e}")
                    nc.sync.dma_start(wtile[:], wd[h, P * e:P * (e + 1), :])
                    wts.append(wtile)
                return wts

            # ---------------- V projection (into packed vA) ----------------
            # e-major waves of 4 open PSUM pair-groups so the PE tracks the
            # x^T/W_v DMA arrival frontier instead of head-of-line blocking
            # on the first group.
            def v_evac(src512, j):
                src = src512.rearrange("p (h q) -> p h q", q=D)
                dst = vA[j].rearrange("p (h q) -> p h q", q=D + 1)
                nc.vector.tensor_copy(dst[:, :, 0:D], src)
                nc.vector.memset(dst[:, :, D:D + 1], 1.0)

            for wave in range(2):
                base = 8 * wave
                pairs = [psp.tile([P, 1024], fdt, tag="sp",
                                  name=f"vpp{wave}_{g}") for g in range(2)]
                sing = [psp.tile([P, 512], fdt, tag="vp",
                                 name=f"vps{wave}_{g}") for g in range(4)]
                units = []          # (psum slice, j)
                for g in range(2):
                    for half in range(2):
                        units.append((
                            pairs[g][:, 512 * half:512 * (half + 1)],
                            base + 2 * g + half))
                for g in range(4):
                    units.append((sing[g][:], base + 4 + g))
                for ps, j in units:
                    nc.tensor.matmul(ps, ones_row[:], bv_sb[:],
                                     start=True, stop=False)
                for e in range(NE):
                    for ps, j in units:
                        nc.tensor.matmul(
                            ps, xts[e][:, P * j:P * (j + 1)], wvs[e][:],
                            start=False, stop=(e == NE - 1))
                for ps, j in units:
                    v_evac(ps, j)

            # ---------------- per-head: Q/K projection + attention --------
            # Software-pipelined: PV groups of chunk c are emitted (drained)
            # during chunk c+1's score matmuls and the next head's
            # projections, so the PE never head-of-line blocks on ScalarE's
            # exp of the chunk it just produced.
            pvq = []

            def drain(n):
                for _ in range(min(n, len(pvq))):
                    pvq.pop(0)()

            AluAdd = mybir.AluOpType.add

            def emit_qk_pair(wts, bias_pt, dst, h, cp, pfx):
                ps = psp.tile([P, 1024], fdt, tag="sp",
                              name=f"{pfx}ps{h}_{cp}")
                for half in range(2):
                    c = 2 * cp + half
                    phs = slice(512 * half, 512 * (half + 1))
                    for e in range(NE):
                        nc.tensor.matmul(
                            ps[:, phs], wts[e][:],
                            xts[e][:, 512 * c:512 * (c + 1)],
                            start=(e == 0), stop=(e == NE - 1))
                for half in range(2):
                    c = 2 * cp + half
                    cs = slice(512 * c, 512 * (c + 1))
                    phs = slice(512 * half, 512 * (half + 1))
                    xs = ep.tile([P, 512], bdt, tag="xs",
                                 name=f"{pfx}xs{h}_{c}")
                    nc.vector.tensor_scalar(xs[:], ps[:, phs],
                                            bias_pt[:, h:h + 1], None,
                                            AluAdd)
                    sw = ep.tile([P, 512], bdt, tag="sw",
                                 name=f"{pfx}sw{h}_{c}")
                    nc.sync.dma_start(sw[0:64, :], xs[64:128, :])
                    nc.sync.dma_start(sw[64:128, :], xs[0:64, :])
                    t1 = ep.tile([P, 512], bdt, tag="t1",
                                 name=f"{pfx}t1_{h}_{c}")
                    nc.vector.tensor_mul(t1[:], xs[:], cos_sb[:, cs])
                    t2 = ep.tile([P, 512], bdt, tag="t2",
                                 name=f"{pfx}t2_{h}_{c}")
                    nc.vector.tensor_mul(t2[:], sw[:], ssin_sb[:, cs])
                    nc.vector.tensor_add(dst[:, cs], t1[:], t2[:])

            def emit_diag(h, c, sel):
                for t in range(4 * c, 4 * c + 4):
                    o = P * (t % 4)
                    w = 512 - o
                    ps_sc = psp.tile([P, w], fdt, tag="vp",
                                     name=f"sc{h}_{c}_{t}")
                    nc.tensor.matmul(
                        ps_sc[:], kTt[:, P * t:P * (t + 1)],
                        qTt[:, 512 * c + o:512 * (c + 1)],
                        start=True, stop=True)
                    et = etd.tile([P, w], bdt, tag=f"etd{t % 4}",
                                  name=f"et{h}_{c}_{t}")
                    nc.scalar.activation(et[:], ps_sc[:], Exp,
                                         scale=SM_SCALE)
                    etm = etd.tile([P, w], bdt, tag=f"etm{t % 4}",
                                   name=f"etm{h}_{c}_{t}", bufs=3)
                    nc.vector.tensor_mul(etm[:], et[:], mask_sb[:, 0:w])
                    sel[t] = (etm, -o)

            def emit_pair(h, c, tp, sel):
                ps_sc = psp.tile([P, 1024], fdt, tag="sp",
                                 name=f"scp{h}_{c}_{tp}")
                for half in range(2):
                    t = 2 * tp + half
                    nc.tensor.matmul(
                        ps_sc[:, 512 * half:512 * (half + 1)],
                        kTt[:, P * t:P * (t + 1)],
                        qTt[:, 512 * c:512 * (c + 1)],
                        start=True, stop=True)
                et = etp.tile([P, 1024], bdt, tag=f"etp{tp}",
                              name=f"etp{h}_{c}_{tp}")
                nc.scalar.activation(et[:], ps_sc[:], Exp, scale=SM_SCALE)
                sel[2 * tp] = (et, 0)
                sel[2 * tp + 1] = (et, 512)

            def make_pv(h, c, jj, sel):
                def go():
                    j = 4 * c + jj
                    po = psp.tile([P, D + 1], fdt, tag="vp",
                                  name=f"po{h}_{j}")
                    for t in range(j + 1):
                        stile, base = sel[t]
                        lo = base + P * jj
                        nc.tensor.matmul(
                            po[:], stile[:, lo:lo + P],
                            vA[t][:, 129 * h:129 * h + 129],
                            start=(t == 0), stop=(t == j))
                    rec = osp.tile([P, 1], fdt, tag="rec",
                                   name=f"rec{h}_{j}")
                    nc.vector.reciprocal(rec[:], po[:, D:D + 1])
                    ot = osp.tile([P, D], fdt, tag="ot", name=f"ot{h}_{j}")
                    nc.vector.tensor_scalar_mul(ot[:], po[:, 0:D], rec[:])
                    nc.sync.dma_start(
                        outd[P * j:P * (j + 1), D * h:D * (h + 1)], ot[:])
                return go

            wqs = load_wh(wqt, 0, "q")
            wks = load_wh(wkt, 0, "k")
            for cp in range(2):
                emit_qk_pair(wqs, bqpt, qTt, 0, cp, "q")
            for cp in range(2):
                emit_qk_pair(wks, bkpt, kTt, 0, cp, "k")

            for h in range(HPC):
                for c in range(NCH):
                    sel = [None] * (4 * c + 4)
                    drain(1)
                    emit_diag(h, c, sel)
                    for tp in range(2 * c):
                        emit_pair(h, c, tp, sel)
                        drain(1)
                    for jj in range(4):
                        pvq.append(make_pv(h, c, jj, sel))
                if h < HPC - 1:
                    wqs = load_wh(wqt, h + 1, "q")
                    wks = load_wh(wkt, h + 1, "k")
                    emit_qk_pair(wqs, bqpt, qTt, h + 1, 0, "q")
                    drain(2)
                    emit_qk_pair(wqs, bqpt, qTt, h + 1, 1, "q")
                    drain(2)
                    emit_qk_pair(wks, bkpt, kTt, h + 1, 0, "k")
                    drain(2)
                    emit_qk_pair(wks, bkpt, kTt, h + 1, 1, "k")
                    drain(2)
                else:
                    drain(len(pvq))
            drain(len(pvq))

    nc.compile()
    return nc


def get_compiled():
    global _compiled
    if _compiled is None:
        _compiled = _build()
    return _compiled


def make_in_maps(logits, Wq, bq, Wk, bk, Wv, bv):
    cosf, ssin = _rope_tables()
    maskm = _mask_tile()
    xts = [np.ascontiguousarray(np.asarray(logits)[b].T).astype(BF16)
           for b in range(B)]

    def permW(Wm, rows):
        # [HPC, E, D]: per-head contiguous transposed+permuted weight blocks
        Wp = np.asarray(Wm)[rows].reshape(HPC, D, E)[:, _PERM, :]
        return np.ascontiguousarray(Wp.transpose(0, 2, 1)).astype(BF16)

    def permb(bvec, rows):
        # [128, HPC] f32: column h = permuted bias of head h
        return np.ascontiguousarray(
            np.asarray(bvec)[rows].reshape(HPC, D)[:, _PERM].T
        ).astype(np.float32)

    in_maps = []
    for core in range(NCORES):
        b, g = divmod(core, 4)
        rows = slice(F * g, F * (g + 1))
        in_maps.append({
            "xt": xts[b],
            "wqt": permW(Wq, rows),
            "wkt": permW(Wk, rows),
            "wvt": np.ascontiguousarray(np.asarray(Wv)[rows].T).astype(BF16),
            "bqd": permb(bq, rows),
            "bkd": permb(bk, rows),
            "bvd": np.asarray(bv)[rows].reshape(1, F).astype(BF16),
            "cosd": cosf,
            "ssind": ssin,
            "maskd": maskm,
        })
    return in_maps


def kernel(logits, Wq, bq, Wk, bk, Wv, bv, **_ignored):
    global LAST_RESULT
    from concourse.bass_utils import run_bass_kernel_spmd

    nc = get_compiled()
    in_maps = make_in_maps(logits, Wq, bq, Wk, bk, Wv, bv)
    res = run_bass_kernel_spmd(nc, in_maps, list(range(NCORES)))
    LAST_RESULT = res
    out = np.empty((B, S, H * D), dtype=np.float32)
    for core in range(NCORES):
        b, g = divmod(core, 4)
        out[b, :, F * g:F * (g + 1)] = res.results[core]["out"]
    return out
